# revision 1
# baseline (speedup 1.0000x reference)
"""Trainium2 Bass kernel for nn_DecoderCell_59742995087471.

Decoder cell: causal self-attention + add&LN, cross-attention over H + add&LN,
single-Linear FFN + add&LN.  B=2, S=T=2048, D=1024, 16 heads x 64.

Sharding (no collectives): 8 cores = 2 batch elements x 4 query-blocks of 512
rows.  Each core redundantly computes the K/V projections for its batch element
(from the full S0 / H, which the host replicates per batch) and runs attention +
FFN + all three LayerNorms for its own 512 query rows.  The causal mask arrives
as data ([2048, 512] multiplicative 0/1, applied post-exp), so the instruction
stream is identical on every core (SPMD).

Layout: activations are kept transposed in SBUF ([d on partitions, rows free]).
All matmul operands are bf16 (fp32 PSUM accumulate); residuals and LN math stay
fp32.  Softmax has no max-subtraction (scores are O(1) for this problem's data
scale: weights ~N(0, 0.02^2), activations ~N(0,1), so |scores/8| < ~4) and the
denominator comes free from a ones-augmented column in the PV matmul.
"""

import numpy as np
import ml_dtypes

import concourse.bass as bass
import concourse.bacc as bacc
import concourse.mybir as mybir
import concourse.tile as tile

F32 = mybir.dt.float32
BF16 = mybir.dt.bfloat16
AF = mybir.ActivationFunctionType
ALU = mybir.AluOpType

B, S, D, H, HD = 2, 2048, 1024, 16, 64
QL = 512          # query rows per core
NC = 8            # cores
DT = D // 128     # 8 d-tiles
KT = S // 128     # 16 key tiles
PAIRS = H // 2    # 8 head pairs
EPS = 1e-5

W_NAMES = ["Wq1", "Wk1", "Wv1", "Wo1", "Wq2", "Wk2", "Wv2", "Wo2", "Wf"]
# par columns (per-partition params, [1024, NPAR] fp32)
PC = {"bq1": 0, "bk1": 1, "bo1": 2, "g1": 3, "b1": 4,
      "bq2": 5, "bk2": 6, "bo2": 7, "g2": 8, "b2": 9,
      "bf": 10, "g3": 11, "b3": 12}
NPAR = 13

BUFS = {
    "xt": 8,     # [128,2048] bf16: x0t -> ht rotation
    "kT": 4,     # [128,2048] bf16: K pairs stream through attn
    "v": 16,     # [128,1040] bf16: V1 -> V2 rotation
    "qT": 8,     # [128,512] bf16 Q^T pairs
    "sb16": 10,  # [128,512] bf16: x0q, s1, s2
    "res": 10,   # [128,512] f32 residual stream generations
    "xpre": 2, "xsq": 2,
    "m": 6,      # mask tiles
    "p": 5,      # [128,1024] bf16 probs
    "o": 8,      # [128,512] bf16 oT pairs
    "w": 10,    # [128,1024] bf16 weights (non-K)
    "wk": 8,    # [128,1024] bf16 K-proj weights (long-lived: filler reads)
    "sm": 3,     # [1,512] f32 smalls
    "smb": 2,    # [128,512] f32 broadcasts
    "rb": 2,     # [64,512] f32
    "t1": 2, "t2": 2,  # [128,512] f32 LN temps
}


def _build_body(nc, tc, d, ctx):
    pools = {}

    def _pool(tag, bufs, space="SBUF"):
        if tag not in pools:
            pools[tag] = ctx.enter_context(
                tc.tile_pool(name=tag, bufs=bufs, space=space))
        return pools[tag]

    # create every pool up front (before any instruction is emitted)
    for tag, bufs in BUFS.items():
        _pool(tag, bufs)
    for dt_ in range(DT):
        _pool(f"par{dt_}", 1)
    for tag in ("ones", "eps"):
        _pool(tag, 1)
    for tag, bufs in (("acc", 2), ("pv", 2), ("sc", 2)):
        _pool("ps_" + tag, bufs, space="PSUM")

    def sbt(shape, dtype, tag):
        return _pool(tag, BUFS[tag]).tile(shape, dtype, tag=tag, name=tag)

    class _PS:
        @staticmethod
        def tile(shape, dtype, tag, bufs, name):
            return _pool("ps_" + tag, bufs, space="PSUM").tile(
                shape, dtype, tag=tag, name=name)
    PS = _PS()

    class _SB:
        @staticmethod
        def tile(shape, dtype, tag, bufs, name):
            return _pool(tag, bufs).tile(shape, dtype, tag=tag, name=name)
    SB = _SB()

    # ---------------- constants / params ----------------
    par_t = []
    for dt_ in range(DT):
        pt = SB.tile([128, NPAR], F32, tag=f"par{dt_}", bufs=1, name=f"par{dt_}")
        nc.sync.dma_start(pt, d["par"][dt_ * 128:(dt_ + 1) * 128, :])
        par_t.append(pt)
    ones_t = SB.tile([128, 1], BF16, tag="ones", bufs=1, name="ones")
    nc.vector.memset(ones_t, 1.0)
    eps_t = SB.tile([1, 1], F32, tag="eps", bufs=1, name="eps")
    nc.vector.memset(eps_t, EPS)

    def pap(dt_, key):
        c = PC[key]
        return par_t[dt_][:, c:c + 1]

    # ---------------- input loads ----------------
    # queue order matters: x0q + Wq1 first so Q1 can start ~9us in; the bulk
    # loads follow and overlap Q1 compute.
    x0q = []
    wq1 = []
    for dt_ in range(DT):
        t = sbt([128, QL], BF16, "sb16")
        nc.sync.dma_start(t, d["x0q"][dt_ * 128:(dt_ + 1) * 128, :])
        x0q.append(t)
        t = sbt([128, D], BF16, "w")
        nc.sync.dma_start(t, d["Wq1"][dt_ * 128:(dt_ + 1) * 128, :])
        wq1.append(t)

    def load_w(name, tag="w"):
        tiles = []
        for dt_ in range(DT):
            t = sbt([128, D], BF16, tag)
            nc.sync.dma_start(t, d[name][dt_ * 128:(dt_ + 1) * 128, :])
            tiles.append(t)
        return tiles

    # ---------------- building blocks ----------------
    def proj_pair_unit(w_t, x_t, out_tile, pair, c0, bias_ap, copy_dve=False):
        """out_tile[:, c0:c0+512] (bf16) = W[:, pair].T @ x[:, c0:c0+512] + bias"""
        acc = PS.tile([128, 512], F32, tag="acc", bufs=2, name="acc")
        for dt_ in range(DT):
            nc.tensor.matmul(acc, w_t[dt_][:, pair * 128:(pair + 1) * 128],
                             x_t[dt_][:, c0:c0 + 512],
                             start=(dt_ == 0), stop=(dt_ == DT - 1))
        if copy_dve:
            # inside attention ACT is the pacing engine - keep copies off it
            nc.vector.tensor_scalar(out_tile[:, c0:c0 + 512], acc, bias_ap,
                                    None, op0=ALU.add)
        else:
            nc.scalar.activation(out_tile[:, c0:c0 + 512], acc, AF.Identity,
                                 bias=bias_ap)

    def v_unit(w_t, x_t, vtile, kt_, half):
        """vtile heads [half*8:(half+1)*8] cols = x[:, kt].T @ W[:, half*512:...]"""
        acc = PS.tile([128, 512], F32, tag="acc", bufs=2, name="acc")
        for dt_ in range(DT):
            nc.tensor.matmul(acc, x_t[dt_][:, kt_ * 128:(kt_ + 1) * 128],
                             w_t[dt_][:, half * 512:(half + 1) * 512],
                             start=(dt_ == 0), stop=(dt_ == DT - 1))
        vv = vtile.rearrange("p (h c) -> p h c", h=H)[:, half * 8:(half + 1) * 8, 0:HD]
        av = acc.rearrange("p (h c) -> p h c", h=8)
        nc.vector.tensor_copy(vv, av)

    def emit_k_pair(wk, x_t, bk_key, pair, copy_dve=False):
        kt_t = sbt([128, S], BF16, "kT")
        for c in range(S // 512):
            proj_pair_unit(wk, x_t, kt_t, pair, c * 512, pap(pair, bk_key),
                           copy_dve=copy_dve)
        return kt_t

    def emit_q_all(wq, xq_t, bq_key):
        qT = []
        for pair in range(PAIRS):
            qt = sbt([128, QL], BF16, "qT")
            proj_pair_unit(wq, xq_t, qt, pair, 0, pap(pair, bq_key))
            qT.append(qt)
        return qT

    def emit_v_all(wv, x_t):
        v_ = []
        for kt_ in range(KT):
            vt = sbt([128, H * (HD + 1)], BF16, "v")
            nc.vector.memset(
                vt.rearrange("p (h c) -> p h c", h=H)[:, :, HD:HD + 1], 1.0)
            for half in range(2):
                v_unit(wv, x_t, vt, kt_, half)
            v_.append(vt)
        return v_

    def make_k(wk, x_t, bk_key, n_upfront):
        kT_ = [emit_k_pair(wk, x_t, bk_key, p) for p in range(n_upfront)]

        def k_filler(pair_done):
            nxt = len(kT_)
            if nxt < PAIRS and nxt <= pair_done + 2:
                kT_.append(emit_k_pair(wk, x_t, bk_key, nxt, copy_dve=True))
        return kT_, k_filler

    def emit_attn(kT_t, v_t, qT_t, mask_d, k_filler=None):
        """Returns 8 assembled oT pair tiles ([128, 512] bf16)."""
        oT_pairs = []
        for pair in range(PAIRS):
            pvs = [PS.tile([HD + 1, QL], F32, tag="pv", bufs=2, name="pv")
                   for _ in range(2)]
            prev = None  # (pT, kt) pending PV
            for kt_ in range(KT):
                if mask_d is not None:
                    mt = sbt([128, QL], BF16, "m")
                    nc.sync.dma_start(mt, mask_d[kt_ * 128:(kt_ + 1) * 128, :])
                psc = PS.tile([128, 2 * QL], F32, tag="sc", bufs=2, name="sc")
                for half in range(2):
                    nc.tensor.matmul(
                        psc[:, half * QL:(half + 1) * QL],
                        kT_t[pair][half * HD:(half + 1) * HD,
                                   kt_ * 128:(kt_ + 1) * 128],
                        qT_t[pair][half * HD:(half + 1) * HD, :],
                        start=True, stop=True)
                pT = sbt([128, 2 * QL], BF16, "p")
                nc.scalar.activation(pT, psc, AF.Exp, scale=0.125)
                if mask_d is not None:
                    for half in range(2):
                        nc.vector.tensor_mul(pT[:, half * QL:(half + 1) * QL],
                                             pT[:, half * QL:(half + 1) * QL], mt)
                if prev is not None:
                    ppT, pkt = prev
                    for half in range(2):
                        h = pair * 2 + half
                        nc.tensor.matmul(
                            pvs[half], v_t[pkt][:, h * (HD + 1):h * (HD + 1) + HD + 1],
                            ppT[:, half * QL:(half + 1) * QL],
                            start=(pkt == 0), stop=(pkt == KT - 1),
                            skip_group_check=True)
                prev = (pT, kt_)
            ppT, pkt = prev
            for half in range(2):
                h = pair * 2 + half
                nc.tensor.matmul(
                    pvs[half], v_t[pkt][:, h * (HD + 1):h * (HD + 1) + HD + 1],
                    ppT[:, half * QL:(half + 1) * QL],
                    start=(pkt == 0), stop=(pkt == KT - 1), skip_group_check=True)
            if k_filler is not None:
                k_filler(pair)
            oT = sbt([128, QL], BF16, "o")
            for half in range(2):
                recip = sbt([1, QL], F32, "sm")
                nc.vector.reciprocal(recip, pvs[half][HD:HD + 1, :])
                rb = sbt([HD, QL], F32, "rb")
                nc.gpsimd.partition_broadcast(rb, recip)
                nc.vector.tensor_mul(oT[half * HD:(half + 1) * HD, :],
                                     pvs[half][0:HD, :], rb)
            oT_pairs.append(oT)
        return oT_pairs

    def emit_out_proj(w_t, in_pairs, bias_key, resid_t):
        """pre[dt] (f32) = W.T @ in_pairs + bias + resid"""
        pre = []
        for m in range(DT):
            acc = PS.tile([128, 512], F32, tag="acc", bufs=2, name="acc")
            for pr in range(PAIRS):
                nc.tensor.matmul(acc, w_t[pr][:, m * 128:(m + 1) * 128],
                                 in_pairs[pr],
                                 start=(pr == 0), stop=(pr == PAIRS - 1))
            t = sbt([128, QL], F32, "res")
            nc.vector.scalar_tensor_tensor(t, acc, pap(m, bias_key), resid_t[m],
                                           op0=ALU.add, op1=ALU.add)
            pre.append(t)
        return pre

    def emit_ln(pre_t, g_key, b_key, want_bf16):
        xb, xq_ = [], []
        for dt_ in range(DT):
            t = sbt([128, QL], BF16, "xpre")
            nc.vector.tensor_copy(t, pre_t[dt_])
            xb.append(t)
            t2_ = sbt([128, QL], BF16, "xsq")
            nc.scalar.square(t2_, pre_t[dt_])
            xq_.append(t2_)
        sx = PS.tile([1, QL], F32, tag="acc", bufs=2, name="acc")
        for dt_ in range(DT):
            nc.tensor.matmul(sx, ones_t, xb[dt_], start=(dt_ == 0),
                             stop=(dt_ == DT - 1), skip_group_check=True)
        sxx = PS.tile([1, QL], F32, tag="acc", bufs=2, name="acc")
        for dt_ in range(DT):
            nc.tensor.matmul(sxx, ones_t, xq_[dt_], start=(dt_ == 0),
                             stop=(dt_ == DT - 1), skip_group_check=True)
        mean = sbt([1, QL], F32, "sm")
        nc.vector.tensor_scalar(mean, sx, 1.0 / D, None, op0=ALU.mult)
        meanb = sbt([128, QL], F32, "smb")
        nc.gpsimd.partition_broadcast(meanb, mean)
        msq = sbt([1, QL], F32, "sm")
        nc.vector.tensor_mul(msq, mean, mean)
        var = sbt([1, QL], F32, "sm")
        nc.vector.scalar_tensor_tensor(var, sxx, 1.0 / D, msq,
                                       op0=ALU.mult, op1=ALU.subtract)
        sd = sbt([1, QL], F32, "sm")
        nc.scalar.activation(sd, var, AF.Sqrt, bias=eps_t)
        rstd = sbt([1, QL], F32, "sm")
        nc.vector.reciprocal(rstd, sd)
        rstdb = sbt([128, QL], F32, "smb")
        nc.gpsimd.partition_broadcast(rstdb, rstd)
        out32, out16 = [], []
        for dt_ in range(DT):
            t1 = sbt([128, QL], F32, "t1")
            nc.vector.tensor_sub(t1, pre_t[dt_], meanb)
            t2_ = sbt([128, QL], F32, "t2")
            nc.vector.tensor_mul(t2_, t1, rstdb)
            o32 = sbt([128, QL], F32, "res")
            nc.vector.tensor_scalar(o32, t2_, pap(dt_, g_key), pap(dt_, b_key),
                                    op0=ALU.mult, op1=ALU.add)
            out32.append(o32)
            if want_bf16:
                o16 = sbt([128, QL], BF16, "sb16")
                nc.vector.tensor_scalar(o16, t2_, pap(dt_, g_key),
                                        pap(dt_, b_key), op0=ALU.mult, op1=ALU.add)
                out16.append(o16)
        return out32, out16

    # ---------------- the decoder cell ----------------
    import os
    stop_after = os.environ.get("KSTOP", "")

    def _early_out(tiles):
        for dt_ in range(DT):
            nc.sync.dma_start(d["out"][dt_ * 128:(dt_ + 1) * 128, :], tiles[dt_])
        return True

    x0t = []
    for dt_ in range(DT):
        t = sbt([128, S], BF16, "xt")
        nc.sync.dma_start(t, d["x0t"][dt_ * 128:(dt_ + 1) * 128, :])
        x0t.append(t)
    wv1 = load_w("Wv1"); wk1 = load_w("Wk1", tag="wk")
    q1 = emit_q_all(wq1, x0q, "bq1")
    v1 = emit_v_all(wv1, x0t)
    k1, kf1 = make_k(wk1, x0t, "bk1", 2)
    if stop_after == "qkv1":
        _early_out(x0r); return

    # ht loads reuse x0t slots (dead after QKV1)
    ht = []
    for dt_ in range(DT):
        t = sbt([128, S], BF16, "xt")
        nc.sync.dma_start(t, d["ht"][dt_ * 128:(dt_ + 1) * 128, :])
        ht.append(t)

    x0r = []
    for dt_ in range(DT):
        t = sbt([128, QL], F32, "res")
        nc.sync.dma_start(t, d["x0r"][dt_ * 128:(dt_ + 1) * 128, :])
        x0r.append(t)
    o1 = emit_attn(k1, v1, q1, d["msk"], k_filler=kf1)
    if stop_after == "attn1":
        _early_out(x0r); return

    wv2 = load_w("Wv2")
    v2 = emit_v_all(wv2, ht)

    wo1 = load_w("Wo1")
    pre1 = emit_out_proj(wo1, o1, "bo1", x0r)
    if stop_after == "wo1":
        _early_out(pre1); return
    s1_32, s1_16 = emit_ln(pre1, "g1", "b1", want_bf16=True)
    if stop_after == "ln1":
        _early_out(s1_32); return

    wk2 = load_w("Wk2", tag="wk"); wq2 = load_w("Wq2")
    k2, kf2 = make_k(wk2, ht, "bk2", 2)
    q2 = emit_q_all(wq2, s1_16, "bq2")
    if stop_after == "qkv2":
        _early_out(s1_32); return

    o2 = emit_attn(k2, v2, q2, None, k_filler=kf2)
    if stop_after == "attn2":
        _early_out(s1_32); return

    wo2 = load_w("Wo2")
    pre2 = emit_out_proj(wo2, o2, "bo2", s1_32)
    s2_32, s2_16 = emit_ln(pre2, "g2", "b2", want_bf16=True)

    wf = load_w("Wf")
    pre3 = emit_out_proj(wf, s2_16, "bf", s2_32)
    s3_32, _ = emit_ln(pre3, "g3", "b3", want_bf16=False)

    for dt_ in range(DT):
        nc.sync.dma_start(d["out"][dt_ * 128:(dt_ + 1) * 128, :], s3_32[dt_])


_CACHE = {}


def build_program():
    if "nc" in _CACHE:
        return _CACHE["nc"]
    nc = bacc.Bacc("TRN2", target_bir_lowering=False, debug=False, num_devices=NC)
    d = {}
    d["x0t"] = nc.dram_tensor("x0t", [D, S], BF16, kind="ExternalInput")
    d["ht"] = nc.dram_tensor("ht", [D, S], BF16, kind="ExternalInput")
    d["x0q"] = nc.dram_tensor("x0q", [D, QL], BF16, kind="ExternalInput")
    d["x0r"] = nc.dram_tensor("x0r", [D, QL], F32, kind="ExternalInput")
    d["msk"] = nc.dram_tensor("msk", [S, QL], BF16, kind="ExternalInput")
    for w in W_NAMES:
        d[w] = nc.dram_tensor(w, [D, D], BF16, kind="ExternalInput")
    d["par"] = nc.dram_tensor("par", [D, NPAR], F32, kind="ExternalInput")
    d["out"] = nc.dram_tensor("out", [D, QL], F32, kind="ExternalOutput")

    from contextlib import ExitStack
    with tile.TileContext(nc) as tc:
        with ExitStack() as ctx:
            _build_body(nc, tc, {k: (v[:] if hasattr(v, "ap") else v)
                                 for k, v in d.items()}, ctx)
    nc.compile()
    _CACHE["nc"] = nc
    return nc


def make_in_maps(inputs):
    """Build the 8 per-core input dicts from the full problem inputs."""
    bf = ml_dtypes.bfloat16
    S0 = np.asarray(inputs["S0"], np.float32)
    Hh = np.asarray(inputs["H"], np.float32)

    par = np.zeros((D, NPAR), np.float32)
    for key, col in PC.items():
        src = {"bq1": "bq1", "bk1": "bk1", "bo1": "bo1", "g1": "ln1_g",
               "b1": "ln1_b", "bq2": "bq2", "bk2": "bk2", "bo2": "bo2",
               "g2": "ln2_g", "b2": "ln2_b", "bf": "bf", "g3": "ln3_g",
               "b3": "ln3_b"}[key]
        par[:, col] = np.asarray(inputs[src], np.float32)
    # bv folds exactly into bo: a = (o + bv) @ Wo + bo = o @ Wo + (bv @ Wo + bo)
    par[:, PC["bo1"]] += np.asarray(inputs["bv1"], np.float32) @ np.asarray(
        inputs["Wo1"], np.float32)
    par[:, PC["bo2"]] += np.asarray(inputs["bv2"], np.float32) @ np.asarray(
        inputs["Wo2"], np.float32)

    ws = {w: np.ascontiguousarray(np.asarray(inputs[w], np.float32)).astype(bf)
          for w in W_NAMES}

    in_maps = []
    for c in range(NC):
        b, j = c // 4, c % 4
        q0 = j * QL
        x0t = np.ascontiguousarray(S0[b].T)
        ht = np.ascontiguousarray(Hh[b].T)
        mask = (np.arange(S)[:, None] <= (q0 + np.arange(QL))[None, :])
        m = {
            "x0t": x0t.astype(bf),
            "ht": ht.astype(bf),
            "x0q": np.ascontiguousarray(x0t[:, q0:q0 + QL]).astype(bf),
            "x0r": np.ascontiguousarray(x0t[:, q0:q0 + QL]),
            "msk": mask.astype(bf),
            "par": par,
        }
        m.update(ws)
        in_maps.append(m)
    return in_maps


def kernel(**inputs) -> np.ndarray:
    from concourse.bass_utils import run_bass_kernel_spmd
    nc = build_program()
    in_maps = make_in_maps(inputs)
    res = run_bass_kernel_spmd(nc, in_maps, list(range(NC)))
    _CACHE["last_results"] = res
    out = np.zeros((B, S, D), np.float32)
    for c in range(NC):
        b, j = c // 4, c % 4
        out[b, j * QL:(j + 1) * QL, :] = res.results[c]["out"].T
    return out

